# revision 9
# baseline (speedup 1.0000x reference)
"""Trainium2 Bass kernel for retrieval_knn (65536 queries x 8192 codes, K=32, D=128).

V2 design:
  - Host builds certified per-leaf candidate sets (Lipschitz lattice bounds on
    the 32nd-NN distance, 5^3 sample lattice, vectorized) for 32-query KD
    leaves; leaves whose set exceeds 240 split their queries (16/8/... rows in
    a 32-row position).  4 leaf positions x 32 rows = one 128-row slot; slots
    grouped by similar list length, padded to a run-uniform L (mult of 8) with
    huge descending sentinels.
  - PE computes d2 directly: block-diagonal stationary [qx,qy,qz,|q|^2,1] per
    leaf against moving [-2cx,-2cy,-2cz,1,|c|^2+eps] -> PSUM (128 x L), f32.
  - Selection per run mode:
      L <= 64 (complement): (L-32)/8 rounds of max8+match_replace(imm=-1e30)
        drop the largest d2; one reciprocal d2->bf16 W (dropped -> -1e-30~0).
      L > 64 (direct): w = recip(d2); 4 rounds max8+match_replace(imm=0);
        W = w - remaining via Pool sub (bf16 out).
    DVE chains are interleaved across slot pairs to stay pipeline-limited.
  - PE transposes W (per 128-chunk), ACT copies W^T PSUM->SBUF, per-leaf code
    matmuls (ncols=32, contraction<=128 per chunk) accumulate out; a
    ones-column in the codes tile yields sum(w) free.  Host normalizes.
  - Pool stages d2 and out PSUM->SBUF; out DMAs once per 4 slots.
"""
import sys
import os

sys.path.insert(0, "/opt/trn_rl_repo")

import numpy as np

K = 32
NCORES = 8
D = 128
NPOS = 4        # 32-row leaf positions per slot
CAPHI = 240     # hard cap for any leaf list (2 chunks of <=128 - 8)
COMP_L = 64     # complement selection for runs with L <= this
RUN = 4         # slots per DMA run
CLAMP = 1e-6    # d2 ground clamp folded into the |c|^2 row
BIGBASE = np.float32(3.0e38)


# ----------------------------------------------------------------------------
# Host: certified candidate sets (vectorized lattice bounds)
# ----------------------------------------------------------------------------

def _kd_nodes(q, nleaf):
    P = q.shape[0]
    nodes = [np.arange(P)]
    while len(nodes) < P // nleaf:
        new = []
        for idx in nodes:
            pts = q[idx]
            ax = int(np.argmax(pts.max(0) - pts.min(0)))
            o = np.argsort(pts[:, ax], kind="stable")
            h = len(idx) // 2
            new.append(idx[o[:h]])
            new.append(idx[o[h:]])
        nodes = new
    return nodes


def _certify(q, cpos, nodes, lat, chunk=64):
    """Per-node certified candidate sets: every query's exact K-NN is inside."""
    cc = (cpos * cpos).sum(1)
    out = [None] * len(nodes)
    for c0 in range(0, len(nodes), chunk):
        nds = nodes[c0:c0 + chunk]
        los = np.stack([q[idx].min(0) for idx in nds])        # (n,3)
        his = np.stack([q[idx].max(0) for idx in nds])
        samples = (los[:, None, :]
                   + lat[None, :, :] * (his - los)[:, None, :])  # (n,S,3)
        n, S, _ = samples.shape
        flat = samples.reshape(-1, 3)
        d2s = (flat * flat).sum(1)[:, None] + cc[None, :] - 2.0 * (flat @ cpos.T)
        d32s = np.sqrt(np.maximum(
            np.partition(d2s, K - 1, 1)[:, K - 1], 0)).reshape(n, S)
        for i, idx in enumerate(nds):
            pts = q[idx]
            dqs = np.sqrt(((pts[:, None, :] - samples[i][None, :, :]) ** 2).sum(-1))
            Rq = (d32s[i][None, :] + dqs).min(1)               # (nq,)
            lo, hi = los[c0 + i - c0], his[c0 + i - c0]
            dbox2 = (np.maximum(np.maximum(lo[None] - cpos, cpos - hi[None]),
                                0) ** 2).sum(-1)
            pre = np.nonzero(dbox2 <= (Rq.max() ** 2))[0]
            d2qc = ((pts[:, None, :] - cpos[pre][None, :, :]) ** 2).sum(-1)
            keep = (d2qc <= (Rq[:, None] ** 2)).any(0)
            out[c0 + i] = pre[keep]
    return out


def _build_leaves(q, cpos):
    gs = np.linspace(0, 1, 5)
    sx, sy, sz = np.meshgrid(gs, gs, gs, indexing="ij")
    lat = np.stack([sx, sy, sz], -1).reshape(-1, 3).astype(np.float32)

    nodes = _kd_nodes(q, 32)
    cands = _certify(q, cpos, nodes, lat)

    leaves = []
    stack = list(zip(nodes, cands))
    while stack:
        idx, cand = stack.pop()
        if len(cand) > CAPHI and len(idx) > 1:
            pts = q[idx]
            ax = int(np.argmax(pts.max(0) - pts.min(0)))
            o = np.argsort(pts[:, ax], kind="stable")
            h = len(idx) // 2
            halves = [idx[o[:h]], idx[o[h:]]]
            hc = _certify(q, cpos, halves, lat)
            stack += list(zip(halves, hc))
            continue
        leaves.append((idx, cand[:CAPHI]))
    return leaves


def _pad8(x):
    return max(40, int((x + 7) // 8) * 8)


# ----------------------------------------------------------------------------
# Packing: leaves -> slots (4 x 32-row positions) -> per-core runs
# ----------------------------------------------------------------------------

def _pack(leaves):
    order = sorted(range(len(leaves)), key=lambda li: -len(leaves[li][1]))
    slots = []  # (slotC, [leaf ids])
    for i in range(0, len(order), NPOS):
        grp = order[i:i + NPOS]
        slots.append((max(len(leaves[li][1]) for li in grp), grp))

    percore = [[] for _ in range(NCORES)]
    for r, slot in enumerate(slots):
        blk, pos = divmod(r, NCORES)
        core = pos if blk % 2 == 0 else NCORES - 1 - pos
        percore[core].append(slot)

    nmax = max(len(pc) for pc in percore)
    nmax = ((nmax + RUN - 1) // RUN) * RUN
    template = []
    assign = [[] for _ in range(NCORES)]
    for j in range(nmax):
        Lj = 40
        for c in range(NCORES):
            if j < len(percore[c]):
                Lj = max(Lj, _pad8(percore[c][j][0]))
        template.append({"L": Lj})
        for c in range(NCORES):
            assign[c].append(percore[c][j][1] if j < len(percore[c]) else [])

    for r0 in range(0, len(template), RUN):
        Lr = max(t["L"] for t in template[r0:r0 + RUN])
        for t in template[r0:r0 + RUN]:
            t["L"] = Lr
    return template, assign


# ----------------------------------------------------------------------------
# Device kernel build
# ----------------------------------------------------------------------------

def _build_nc(template, lens):
    import concourse.bass as bass
    import concourse.mybir as mybir
    import concourse.tile as tile_mod
    from concourse.tile import TileContext
    from concourse.vector_clock import ScopedClock
    from concourse.masks import make_identity

    def _split_drain_and_barrier(self, tick_clock, wait_clock):
        nc = self.nc
        carriers = [nc.sync.nop(nofuse=True) for _ in range(40)]
        drain_inst = nc.sync.drain()
        wait_clock.add_sem_waits(drain_inst.ins, ScopedClock({None: tick_clock.global_clock}))
        si = drain_inst.ins.sync_info
        waits = list(si.on_wait or [])
        if len(waits) > 1:
            extra = waits[:-1]
            si.on_wait = waits[-1:]
            for i, w in enumerate(extra):
                c = carriers[i]
                csi = c.ins.sync_info
                if csi is None:
                    c.ins.sync_info = mybir.SyncInfo(on_wait=[w], on_update=[])
                else:
                    csi.on_wait = (csi.on_wait or []) + [w]
        nc.all_engine_barrier()
        popped = nc._tile_sem_poison_stack.pop()
        assert popped is self._sem_poison
        nc.clear_and_free_semaphores(list(self.sems.allocated().values()))
        nc.all_engine_barrier()

    tile_mod.TileContext._drain_and_barrier = _split_drain_and_barrier

    nslots = len(template)
    nc = bass.Bass(trn_type="TRN2")
    f32 = mybir.dt.float32
    bf16 = mybir.dt.bfloat16
    CROWS = 5 * NPOS

    io_d = nc.dram_tensor("iobuf", [lens["io"]], f32, kind="ExternalInput")
    cod_d = nc.dram_tensor("cods", [lens["cod"]], bf16, kind="ExternalInput")
    out_d = nc.dram_tensor("out", [nslots // RUN, 128, RUN * 129], f32,
                           kind="ExternalOutput")

    io_off, cod_off = [0], [0]
    for r0 in range(0, nslots, RUN):
        L = template[r0]["L"]
        nch = 1 if L <= 128 else 2
        io_off.append(io_off[-1] + CROWS * RUN * (128 + L))
        cod_off.append(cod_off[-1] + min(L, 128) * nch * RUN * NPOS * 129)

    Lmax = max(t["L"] for t in template)
    Lmax1 = min(Lmax, 128)   # chunk-1 width cap

    with TileContext(nc) as tc:
        with (
            tc.tile_pool(name="con", bufs=1) as con,
            tc.tile_pool(name="ios", bufs=3) as ios,
            tc.tile_pool(name="wks", bufs=4) as wks,
            tc.tile_pool(name="osb", bufs=2) as osb,
            tc.tile_pool(name="pd2", bufs=3, space="PSUM") as pd2,
            tc.tile_pool(name="pwt", bufs=3, space="PSUM") as pwt,
            tc.tile_pool(name="pou", bufs=2, space="PSUM") as pou,
        ):
            identb = con.tile([128, 128], bf16)
            make_identity(nc, identb)

            nruns = nslots // RUN
            run_tiles = {}

            def dma_in(ri):
                L = template[ri * RUN]["L"]
                nch = 1 if L <= 128 else 2
                io_f = ios.tile([CROWS, RUN * (128 + Lmax)], f32, tag="io")
                cod_f = ios.tile([128, nch * RUN * NPOS * 129], bf16,
                                 tag="cod%d" % nch)
                nc.sync.dma_start(
                    out=io_f[:, :RUN * (128 + L)],
                    in_=io_d[io_off[ri]:io_off[ri + 1]].rearrange(
                        "(p x) -> p x", p=CROWS))
                nc.sync.dma_start(
                    out=cod_f[:min(L, 128), :],
                    in_=cod_d[cod_off[ri]:cod_off[ri + 1]].rearrange(
                        "(p x) -> p x", p=min(L, 128)))
                run_tiles[ri] = (io_f, cod_f)

            dma_in(0)
            for ri in range(nruns):
                if ri + 1 < nruns:
                    dma_in(ri + 1)
                io_f, cod_f = run_tiles.pop(ri)
                r0 = ri * RUN
                L = template[r0]["L"]
                comp = L <= COMP_L
                nch = 1 if L <= 128 else 2
                nr = (L - 32) // 8 if comp else 4
                seg = 128 + L
                sl = list(range(RUN))

                o_sb = osb.tile([128, RUN * 129], f32, tag="osb")
                d2p, d2s, wfl, Wt, mxb, wtp, wts = {}, {}, {}, {}, {}, {}, {}
                for s in sl:
                    d2p[s] = pd2.tile([128, Lmax1 * 2 if Lmax > 128 else Lmax1],
                                      f32, tag="d2p", name="d2p%d" % s)
                    nc.tensor.matmul(
                        d2p[s][:, :L],
                        io_f[:, s * seg:s * seg + 128],
                        io_f[:, s * seg + 128:s * seg + 128 + L],
                        start=True, stop=True)
                for s in sl:
                    d2s[s] = wks.tile([128, Lmax], f32, tag="d2s",
                                      name="d2s%d" % s)
                    nc.gpsimd.tensor_copy(out=d2s[s][:, :L], in_=d2p[s][:, :L])
                    mxb[s] = wks.tile([128, 8], f32, tag="mxb",
                                      name="mxb%d" % s)
                if comp:
                    for j in range(nr):
                        for s in sl:
                            nc.vector.max(out=mxb[s], in_=d2s[s][:, :L])
                        for s in sl:
                            nc.vector.match_replace(
                                out=d2s[s][:, :L], in_to_replace=mxb[s],
                                in_values=d2s[s][:, :L], imm_value=-1.0e30)
                    for s in sl:
                        Wt[s] = wks.tile([128, Lmax], bf16, tag="W",
                                         name="W%d" % s)
                        with nc.allow_low_precision(reason="bf16 weights"):
                            nc.vector.reciprocal(out=Wt[s][:, :L],
                                                 in_=d2s[s][:, :L])
                else:
                    for s in sl:
                        wfl[s] = wks.tile([128, Lmax], f32, tag="wfl",
                                          name="wfl%d" % s)
                        nc.vector.reciprocal(out=wfl[s][:, :L],
                                             in_=d2s[s][:, :L])
                    for j in range(nr):
                        for s in sl:
                            nc.vector.max(out=mxb[s],
                                          in_=wfl[s][:, :L] if j == 0
                                          else d2s[s][:, :L])
                        for s in sl:
                            nc.vector.match_replace(
                                out=d2s[s][:, :L], in_to_replace=mxb[s],
                                in_values=wfl[s][:, :L] if j == 0
                                else d2s[s][:, :L],
                                imm_value=0.0)
                    for s in sl:
                        Wt[s] = wks.tile([128, Lmax], bf16, tag="W",
                                         name="W%d" % s)
                        nc.gpsimd.tensor_sub(out=Wt[s][:, :L],
                                             in0=wfl[s][:, :L],
                                             in1=d2s[s][:, :L])
                for s in sl:
                    wtp[s] = pwt.tile([128, 2 * 128 if Lmax > 128 else 128],
                                      bf16, tag="wtp", name="wtp%d" % s)
                    for ch in range(nch):
                        cw = min(128, L - ch * 128)
                        nc.tensor.transpose(
                            wtp[s][:cw, ch * 128:ch * 128 + 128],
                            Wt[s][:, ch * 128:ch * 128 + cw], identb)
                for s in sl:
                    wts[s] = wks.tile([128, 2 * 128 if Lmax > 128 else 128],
                                      bf16, tag="wts", name="wts%d" % s)
                    nc.scalar.copy(out=wts[s][:min(L, 128), :nch * 128],
                                   in_=wtp[s][:min(L, 128), :nch * 128])
                for p0 in range(0, RUN, 2):
                    o_ps = pou.tile([128, 2 * 129], f32, tag="ops",
                                    name="ops%d" % p0)
                    for s in (p0, p0 + 1):
                        par = s - p0
                        for k in range(NPOS):
                            r0q = k * 32
                            for ch in range(nch):
                                cw = min(128, L - ch * 128)
                                nc.tensor.matmul(
                                    o_ps[r0q:r0q + 32, par * 129:(par + 1) * 129],
                                    wts[s][:cw, ch * 128 + r0q:ch * 128 + r0q + 32],
                                    cod_f[:cw, ((ch * RUN + s) * NPOS + k) * 129:
                                          ((ch * RUN + s) * NPOS + k + 1) * 129],
                                    start=(ch == 0), stop=(ch == nch - 1),
                                    tile_position=(0, r0q))
                    nc.scalar.copy(out=o_sb[:, p0 * 129:(p0 + 2) * 129],
                                   in_=o_ps)
                nc.sync.dma_start(out=out_d[ri], in_=o_sb)

    n = 0
    for f in nc.m.functions:
        for b in f.blocks:
            out = []
            for inst in b.instructions:
                si = inst.sync_info
                waits = list(si.on_wait) if si and si.on_wait else []
                if len(waits) > 1:
                    extra, keep = waits[:-1], waits[-1:]
                    si.on_wait = keep
                    for w in extra:
                        nop = mybir.InstNoOp(name=f"I-wsplit-{n}", ins=[], outs=[])
                        n += 1
                        nop.engine = inst.engine
                        nop.sync_info = mybir.SyncInfo(on_wait=[w], on_update=[])
                        out.append(nop)
                out.append(inst)
            b.instructions = out
    return nc


# ----------------------------------------------------------------------------
# Host buffer packing
# ----------------------------------------------------------------------------

def _fill_core(core, template, assign, leaves, q, qq, cpos_aug, codes_aug,
               lens):
    io = np.zeros(lens["io"], np.float32)
    cod = np.zeros(lens["cod"], "bfloat16")
    meta = []
    io_pos = 0
    cod_pos = 0
    nslots = len(template)
    CROWS = 5 * NPOS
    for r0 in range(0, nslots, RUN):
        L = template[r0]["L"]
        nch = 1 if L <= 128 else 2
        L1 = min(L, 128)
        iobuf = np.zeros((CROWS, RUN, 128 + L), np.float32)
        # codes layout: [chunk, slot, pos] tiles of (L1 x 129)
        codbuf = np.zeros((L1, nch, RUN, NPOS, 129), "bfloat16")
        for p in range(RUN):
            s = r0 + p
            grp = assign[core][s]
            stat = iobuf[:, p, :128]
            coor = iobuf[:, p, 128:]
            for k in range(NPOS):
                coor[5 * k + 3, :] = 1.0
                coor[5 * k + 4, :] = BIGBASE * (1.0 - np.arange(L) * 2.0 ** -12)
                stat[5 * k + 3, k * 32:(k + 1) * 32] = 1.0
            for k, li in enumerate(grp):
                qidx, cand = leaves[li]
                nq, ncd = len(qidx), len(cand)
                r0q = k * 32
                cols = slice(r0q, r0q + nq)
                stat[5 * k + 0:5 * k + 3, cols] = q[qidx].T
                stat[5 * k + 3, cols] = qq[qidx]
                stat[5 * k + 4, cols] = 1.0
                coor[5 * k:5 * k + 5, :ncd] = cpos_aug[cand].T
                for ch in range(nch):
                    sl = cand[ch * 128:ch * 128 + 128]
                    codbuf[:len(sl), ch, p, k, :] = codes_aug[sl]
                meta.append((s, r0q, nq, qidx))
        io[io_pos:io_pos + iobuf.size] = iobuf.reshape(-1)
        cod[cod_pos:cod_pos + codbuf.size] = codbuf.reshape(-1)
        io_pos += iobuf.size
        cod_pos += codbuf.size
    assert io_pos == lens["io"] and cod_pos == lens["cod"], (io_pos, cod_pos)
    return {"iobuf": io, "cods": cod}, meta


def prepare(indices, query_points, codes_position, codes):
    b = int(np.asarray(indices).reshape(-1)[0])
    q = np.asarray(query_points, np.float32)[0]
    cpos = np.asarray(codes_position, np.float32)[b]
    cds = np.asarray(codes, np.float32)[b]
    P = q.shape[0]

    leaves = _build_leaves(q, cpos)
    template, assign = _pack(leaves)
    nslots = len(template)

    qq = (q * q).sum(1).astype(np.float32)
    cc = (cpos * cpos).sum(1).astype(np.float32) + np.float32(CLAMP)
    cpos_aug = np.concatenate(
        [-2.0 * cpos, np.ones((cpos.shape[0], 1), np.float32),
         cc[:, None]], 1).astype(np.float32)
    codes_aug = np.concatenate(
        [cds, np.ones((cds.shape[0], 1), np.float32)], 1).astype("bfloat16")

    lens = {"io": 0, "cod": 0}
    for r0 in range(0, nslots, RUN):
        L = template[r0]["L"]
        nch = 1 if L <= 128 else 2
        lens["io"] += 5 * NPOS * RUN * (128 + L)
        lens["cod"] += min(L, 128) * nch * RUN * NPOS * 129

    in_maps, metas = [], []
    for core in range(NCORES):
        m, meta = _fill_core(core, template, assign, leaves, q, qq,
                             cpos_aug, codes_aug, lens)
        in_maps.append(m)
        metas.append(meta)

    nc = _build_nc(template, lens)
    slot_L = np.array([t["L"] for t in template])
    return {"nc": nc, "in_maps": in_maps, "meta": metas, "P": P,
            "slot_C": slot_L, "template": template}


def assemble(prep, results):
    out = np.zeros((prep["P"], D), np.float32)
    for core in range(NCORES):
        o = results[core]["out"]  # (nruns, 128, RUN*129)
        for s, r0q, nq, qidx in prep["meta"][core]:
            blk = o[s // RUN, r0q:r0q + nq,
                    (s % RUN) * 129:(s % RUN) * 129 + 129]
            out[qidx] = blk[:, :128] / blk[:, 128:129]
    return out


def kernel(indices, query_points, codes_position, codes):
    from concourse.bass_utils import run_bass_kernel_spmd

    prep = prepare(indices, query_points, codes_position, codes)
    res = run_bass_kernel_spmd(prep["nc"], prep["in_maps"],
                               core_ids=list(range(NCORES)))
    return assemble(prep, res.results)
